# revision 1
# baseline (speedup 1.0000x reference)
"""Trainium2 Bass kernel for nn_Cross_attention_3 (sparse_attention).

Sharding: the (D, H*W) plane is unfolded into 9x9 patches; D=72 gives 8
patch-row blocks of 9 rows — exactly one per NeuronCore.  The only
cross-core dependency is the AdaptiveAvgPool over the patch axis
(bins of 128 patches straddle core boundaries); cores exchange 8-patch
half-block partial sums via a 3.4MB AllGather.

The two MLP linears have no nonlinearity between them, so they collapse
into a single 81x81 matrix; the conv bias rides along as an 82nd
contraction row whose rhs holds b[c].  The 1x1x1 conv is computed with
the patch data as the matmul's stationary operand, so its output lands
directly in (patch-element, channel) layout — the transpose the rest of
the pipeline needs comes for free.  Patches are packed in slot-halves
(slot 0 = patches 0..647, slot 1 = 648..1295) so pooling windows and
attention output runs stay contiguous.
"""

import os
import sys

import numpy as np

try:
    import ml_dtypes
except ImportError:
    ml_dtypes = None

try:
    import concourse.bacc as _  # noqa: F401
except ImportError:  # container default path
    sys.path.insert(0, "/opt/trn_rl_repo")

import concourse.bacc as bacc
import concourse.mybir as mybir
from concourse.bass_utils import run_bass_kernel_spmd
from concourse.tile import TileContext

P = 9
P2 = 81
C = 64
D = 72
H = W = 108
HW = H * W
NCORES = 8
LLOC = HW // P   # 1296 patches per core
LP = LLOC // 2   # 648 patch pairs per core (slot halves)

NLP = 24                     # chunk size in pairs, both passes
NCH_A = LP // NLP            # 27 chunks
RING_B = 216                 # pass-B ring, in pairs
NRING = LP // RING_B         # 3
SUB_B = 24                   # pass-B subchunk, in pairs
NHALF = LP // 8              # 81 half-blocks (8 patches) per slot

F32 = mybir.dt.float32
BF16 = mybir.dt.bfloat16

_cache = {}


def _build_nc():
    nc = bacc.Bacc(None, target_bir_lowering=False, debug=False)
    xp_d = nc.declare_dram_parameter("xp", [128, LP, P2], BF16, isOutput=False)
    yp_d = nc.declare_dram_parameter("yp", [128, LP, P2], BF16, isOutput=False)
    wi_d = nc.declare_dram_parameter("wi", [128, 128], BF16, isOutput=False)
    wf_d = nc.declare_dram_parameter("wf", [128, 128], BF16, isOutput=False)
    wm_d = nc.declare_dram_parameter("wm", [82, P2], BF16, isOutput=False)
    bi_d = nc.declare_dram_parameter("bi", [1, SUB_B * 128], BF16, isOutput=False)
    bf_d = nc.declare_dram_parameter("bf", [1, SUB_B * 128], BF16, isOutput=False)
    out_d = nc.declare_dram_parameter("out", [P2, C, 2 * LP], BF16, isOutput=True)

    # per-slot 8-patch half-block sums; gathered across cores
    gath_d = nc.dram_tensor("gath", [NCORES, P2, C, NHALF], F32,
                            addr_space="Shared")

    with nc.allow_low_precision("bf16 compute pipeline"), TileContext(nc) as tc:
        with (
            tc.tile_pool(name="const", bufs=1) as constp,
            tc.tile_pool(name="psconv", bufs=4, space="PSUM") as psconv,
            tc.tile_pool(name="psmlp", bufs=2, space="PSUM") as psmlp,
            tc.tile_pool(name="dram", bufs=1, space="DRAM") as dramp,
        ):
            wi_sb = constp.tile([128, 128], BF16, tag="wi")
            wf_sb = constp.tile([128, 128], BF16, tag="wf")
            wm_sb = constp.tile([82, P2], BF16, tag="wm")
            pooled = constp.tile([P2, C, P2], BF16, tag="pooled")
            h_dram = dramp.tile([P2, C, NHALF], F32)
            nc.sync.dma_start(out=wi_sb[:, :], in_=wi_d[:, :])
            nc.sync.dma_start(out=wf_sb[:, :], in_=wf_d[:, :])
            nc.sync.dma_start(out=wm_sb[:, :], in_=wm_d[:, :])

            def conv_mlp(stp, mlq, nlp, src_d, w_sb, b_d, lp0, dst, dst_lp0,
                         dst_nlp, act_ix):
                """conv+MLP+lrelu for NLP pairs starting at pair lp0 of src_d.
                dst: (81, dst_nlp, 128) bf16 tile, (lp, sc)-major, written at
                lp offset dst_lp0."""
                st = stp.tile([128, nlp, P2], BF16, tag="stage")
                nc.sync.dma_start(out=st[:, :, :], in_=src_d[:, lp0:lp0 + nlp, :])
                ms = mlq.tile([82, nlp, 128], BF16, tag="ms")
                nc.sync.dma_start(
                    out=ms[81:82, :, :].rearrange("p a b -> p (a b)"),
                    in_=b_d[:, 0:nlp * 128],
                )
                # conv: 4 pairs per PSUM bank, N=128 each; one contiguous
                # evict per bank into ms (lp-major); DVE 2/3, ACT 1/3
                for pb in range(nlp // 4):
                    ps = psconv.tile([P2, 512], F32, tag="psc")
                    for j in range(4):
                        nc.tensor.matmul(
                            ps[0:P2, 128 * j:128 * (j + 1)],
                            st[:, 4 * pb + j, :], w_sb[:, :],
                            start=True, stop=True,
                        )
                    d = ms[0:P2, 4 * pb:4 * pb + 4, :].rearrange("p a b -> p (a b)")
                    nc.vector.tensor_copy(d, ps[0:P2, 0:512])
                # MLP (+bias row) and LeakyReLU on ACT; 512 cols = 4 lp
                flat = ms[:, :, :].rearrange("p a b -> p (a b)")
                for q in range(nlp // 4):
                    mp = psmlp.tile([P2, 512], F32, tag="psm")
                    nc.tensor.matmul(
                        mp[0:P2, :], wm_sb[:, :], flat[:, 512 * q:512 * (q + 1)],
                        start=True, stop=True,
                    )
                    dq = dst[0:P2, dst_lp0 + 4 * q:dst_lp0 + 4 * q + 4,
                             :].rearrange("p a b -> p (a b)")
                    nc.scalar.activation(
                        dq, mp[0:P2, :],
                        mybir.ActivationFunctionType.Prelu, alpha=0.2,
                    )

            # ---------------- pass A: fea (y) + pooled ----------------------
            with (
                tc.tile_pool(name="stageA", bufs=3) as stagep,
                tc.tile_pool(name="mlpsA", bufs=2) as mlpp,
                tc.tile_pool(name="feaout", bufs=2) as feap,
                tc.tile_pool(name="pooltmp", bufs=2) as ptp,
                tc.tile_pool(name="hsb", bufs=1) as hsbp,
                tc.tile_pool(name="comb", bufs=2) as combp,
            ):
                # h_sb: 8-patch half-block sums, (81, 162, 64): dim1 =
                # slot*81 + h, dim2 = c
                h_sb = hsbp.tile([P2, 2 * NHALF, C], F32, tag="hsb")
                s_sb = hsbp.tile([P2, C, NHALF], F32, tag="ssb")
                for ch in range(NCH_A):
                    fea = feap.tile([P2, NLP, 128], BF16, tag="fea")
                    conv_mlp(stagep, mlpp, NLP, yp_d, wf_sb, bf_d, ch * NLP,
                             fea, 0, NLP, ch)
                    # pairwise tree over lp: 24 -> 12 -> 6 -> 3 half-sums
                    t1 = ptp.tile([P2, 12, 128], BF16, tag="t1")
                    f2 = fea[0:P2, :, :].rearrange("p (a two) b -> p a two b", two=2)
                    nc.vector.tensor_tensor(
                        t1[0:P2, :, :], f2[:, :, 0, :], f2[:, :, 1, :],
                        mybir.AluOpType.add,
                    )
                    t2 = ptp.tile([P2, 6, 128], BF16, tag="t2")
                    t1s = t1[0:P2, :, :].rearrange("p (a two) b -> p a two b", two=2)
                    nc.vector.tensor_tensor(
                        t2[0:P2, :, :], t1s[:, :, 0, :], t1s[:, :, 1, :],
                        mybir.AluOpType.add,
                    )
                    t3 = ptp.tile([P2, 3, 128], F32, tag="t3")
                    t2s = t2[0:P2, :, :].rearrange("p (a two) b -> p a two b", two=2)
                    nc.vector.tensor_tensor(
                        t3[0:P2, :, :], t2s[:, :, 0, :], t2s[:, :, 1, :],
                        mybir.AluOpType.add,
                    )
                    # scatter the 3 half-sums per slot into h_sb
                    for slot in range(2):
                        nc.vector.tensor_copy(
                            h_sb[0:P2, NHALF * slot + 3 * ch:
                                 NHALF * slot + 3 * ch + 3, :],
                            t3[0:P2, :, 64 * slot:64 * slot + 64],
                        )
                # merge halves into 81 16-patch blocks (S), c-major for the
                # gather: S[e, c, s]
                hv = h_sb[0:P2, :, :]
                # s in [0, 40): slot0 pairs (2s, 2s+1)
                e0 = hv[:, 0:80, :].rearrange("p (a two) c -> p a two c", two=2)
                nc.vector.tensor_tensor(
                    s_sb[0:P2, :, 0:40].rearrange("p c s -> p s c"),
                    e0[:, :, 0, :], e0[:, :, 1, :], mybir.AluOpType.add,
                )
                # s = 40: slot0 h=80 + slot1 h=0
                nc.vector.tensor_tensor(
                    s_sb[0:P2, :, 40:41].rearrange("p c s -> p s c"),
                    hv[:, 80:81, :], hv[:, 81:82, :], mybir.AluOpType.add,
                )
                # s in [41, 81): slot1 pairs (1+2t, 2+2t)
                e1 = hv[:, 82:162, :].rearrange("p (a two) c -> p a two c", two=2)
                nc.vector.tensor_tensor(
                    s_sb[0:P2, :, 41:81].rearrange("p c s -> p s c"),
                    e1[:, :, 0, :], e1[:, :, 1, :], mybir.AluOpType.add,
                )
                nc.gpsimd.dma_start(out=h_dram[:, :, :], in_=s_sb[:, :, :])
                nc.gpsimd.collective_compute(
                    "AllGather",
                    mybir.AluOpType.bypass,
                    replica_groups=[list(range(NCORES))],
                    ins=[h_dram[:, :, :]],
                    outs=[gath_d[:, :, :, :]],
                )
                # bins of 128 = 8 consecutive global 16-blocks (s_g = 81k + s)
                for cc in range(8):
                    tcb = combp.tile([P2, 8, NCORES * NHALF], F32, tag="tcb")
                    for k in range(NCORES):
                        nc.gpsimd.dma_start(
                            out=tcb[0:P2, :, NHALF * k:NHALF * (k + 1)],
                            in_=gath_d[k, :, 8 * cc:8 * (cc + 1), :],
                        )
                    pr = combp.tile([P2, 8, P2], F32, tag="pr")
                    nc.vector.tensor_reduce(
                        pr[0:P2, :, :],
                        tcb[0:P2, :, :].rearrange("p c (j m) -> p c j m", m=8),
                        mybir.AxisListType.X,
                        mybir.AluOpType.add,
                    )
                    nc.vector.tensor_scalar_mul(
                        pooled[0:P2, 8 * cc:8 * (cc + 1), :], pr[0:P2, :, :],
                        1.0 / 128.0,
                    )

            # ---------------- pass B: img (x) + attention -------------------
            with (
                tc.tile_pool(name="stageB", bufs=3) as stagepB,
                tc.tile_pool(name="mlpsB", bufs=2) as mlppB,
                tc.tile_pool(name="imgring", bufs=1) as imgp,
                tc.tile_pool(name="attev", bufs=4) as attevp,
                tc.tile_pool(name="psatt", bufs=2, space="PSUM") as psatt,
            ):
                for ring in range(NRING):
                    img = imgp.tile([P2, RING_B, 128], BF16, tag="img")
                    for sub in range(RING_B // SUB_B):
                        conv_mlp(
                            stagepB, mlppB, SUB_B, xp_d, wi_sb, bi_d,
                            ring * RING_B + sub * SUB_B, img, sub * SUB_B,
                            RING_B, sub,
                        )
                    lp0 = ring * RING_B
                    ncols = RING_B * 2
                    for c in range(C):
                        ap = psatt.tile([P2, 512], F32, tag="psa")
                        # rhs cols (slot, lp): l = 648*slot + lp0 + lp
                        rhs = img[0:P2, :, :].rearrange(
                            "p l (s c) -> p c s l", s=2
                        )[:, c:c + 1, :, :]
                        nc.tensor.matmul(
                            ap[0:P2, 0:ncols], pooled[:, c:c + 1, :], rhs,
                            start=True, stop=True,
                        )
                        ev = attevp.tile([P2, 2, RING_B], BF16, tag="attev")
                        src = ap[0:P2, 0:ncols].rearrange("p (s l) -> p s l", s=2)
                        nc.scalar.copy(ev[0:P2, :, :], src)
                        # out[e, c, 648*slot + lp0 : +RING_B]
                        dstap = out_d[0:P2, c:c + 1, :].rearrange(
                            "p o (s l) -> p o s l", s=2
                        )[:, :, :, lp0:lp0 + RING_B]
                        nc.sync.dma_start(out=dstap, in_=ev[0:P2, :, :])
    nc.compile()
    return nc


def _host_prep(x, y, w_img, b_img, w_fea, b_fea, w1, w2):
    f32 = np.float32
    bf16 = ml_dtypes.bfloat16
    weff = (w2.astype(np.float64) @ w1.astype(np.float64))  # (81, 81)
    wm = np.concatenate([weff.T, weff.sum(axis=1)[None, :]], axis=0)
    wm = wm.astype(f32).astype(bf16)

    def pairw(w):
        blk = np.zeros((128, 128), dtype=f32)
        blk[0:64, 0:64] = w.T
        blk[64:128, 64:128] = w.T
        return blk.astype(bf16)

    wi = pairw(w_img.astype(f32))
    wf = pairw(w_fea.astype(f32))
    # bias row in (sc, lp)-major order: value b[sc % 64] repeated NLP times
    bi = np.tile(np.concatenate([b_img, b_img]).astype(f32), SUB_B)[None, :]
    bf_ = np.tile(np.concatenate([b_fea, b_fea]).astype(f32), SUB_B)[None, :]
    bi = bi.astype(bf16)
    bf_ = bf_.astype(bf16)

    def unf_pairs(t):  # (1, 64, 72, 108, 108) -> per-core (128, 648, 81)
        u = np.ascontiguousarray(
            t.reshape(C, NCORES, P, LLOC, P).transpose(1, 0, 3, 2, 4)
        ).reshape(NCORES, C, LLOC, P2)
        out = []
        for k in range(NCORES):
            v = u[k].reshape(C, 2, LP, P2).transpose(1, 0, 2, 3)  # slot-halves
            out.append(np.ascontiguousarray(v.reshape(128, LP, P2)).astype(bf16))
        return out

    xps = unf_pairs(np.asarray(x, dtype=f32))
    yps = unf_pairs(np.asarray(y, dtype=f32))
    shared = {"wi": wi, "wf": wf, "wm": wm, "bi": bi, "bf": bf_}
    return [dict(shared, xp=xps[k], yp=yps[k]) for k in range(NCORES)]


def kernel(x, y, w_img, b_img, w_fea, b_fea, w1, w2):
    if "nc" not in _cache:
        _cache["nc"] = _build_nc()
    nc = _cache["nc"]
    in_maps = _host_prep(x, y, w_img, b_img, w_fea, b_fea, w1, w2)
    trace = bool(os.environ.get("KERNEL_TRACE"))
    res = run_bass_kernel_spmd(
        nc, in_maps, list(range(NCORES)), trace=trace
    )
    _cache["last_result"] = res
    out = np.empty((1, C, D, H, W), dtype=np.float32)
    ov = out.reshape(C, D, HW)
    for k in range(NCORES):
        # out_d is (81, 64, 1296) with l = 648*slot + lp (already global l)
        att = res.results[k]["out"].astype(np.float32).transpose(1, 2, 0)
        blk = att.reshape(C, LLOC, P, P).transpose(0, 2, 1, 3).reshape(C, P, HW)
        ov[:, P * k:P * (k + 1), :] = blk
    return out



# revision 9
# speedup vs baseline: 1.3995x; 1.3995x over previous
"""Trainium2 Bass kernel for nn_Cross_attention_3 (sparse_attention).

Two passes over 8 cores (uniform SPMD program):

Pass A (fea, y): each core owns a bin-ALIGNED span of 1408 global patches
(core k<7: patches [1280k, 1280k+1280) + 128 zero pads; core 7: the last
1408).  conv+MLP+lrelu, then 8-patch block sums and three fixed grouped
reduces give this core's 11 local 128-patch pooling bins.  Zero pads
contribute exactly 0 (zero data + zero staged conv-bias columns, and
lrelu(0)=0).  The 1/128 pool mean is folded into the fea MLP weight
(lrelu is positively homogeneous).  A 114KB-per-core bf16 AllGather
exchanges the bins; 8 fixed readback DMAs assemble the global pooled
[81, 81bins, 64c] - identical constants on every core.

Pass B (img, x): original even D-row sharding (1296 patches/core, no
pads), conv+MLP+lrelu into 3 rings of 216 pairs, then per-channel
attention matmuls against pooled.  The collective flies under ring 0/1
conv.  All PSUM evictions are round-robined across the Activation,
Vector and Pool engines (1024-col 2-bank evicts) so the PE never waits
on a single eviction engine.
"""

import os
import sys

import numpy as np

try:
    import ml_dtypes
except ImportError:
    ml_dtypes = None

try:
    import concourse.bacc as _  # noqa: F401
except ImportError:  # container default path
    sys.path.insert(0, "/opt/trn_rl_repo")

import concourse.bacc as bacc
import concourse.mybir as mybir
from concourse.bass_utils import run_bass_kernel_spmd
from concourse.tile import TileContext

P = 9
P2 = 81
C = 64
D = 72
H = W = 108
HW = H * W
NCORES = 8
L = (D // P) * (HW // P)   # 10368 global patches
NBIN = 81                  # global 128-patch pooling bins

# pass A (fea): bin-aligned spans, 1408 staged patches = 704 pairs
LPA = 704
NLPA = 16
NCHA = LPA // NLPA         # 44 chunks
NREAL = 1280               # real patches for cores 0..6 (core 7: 1408)

# pass B (img): even sharding, 1296 patches = 648 pairs
LLOC = HW // P             # 1296 patches per core
LPB = LLOC // 2            # 648
NLPB = 24
NCHB = LPB // NLPB         # 27 chunks
RING = 9                   # chunks per att ring
NRING = NCHB // RING       # 3
RB = RING * NLPB           # 216 pairs per ring

F32 = mybir.dt.float32
BF16 = mybir.dt.bfloat16
AX = mybir.AxisListType.X
ADD = mybir.AluOpType.add
MULT = mybir.AluOpType.mult
MAX = mybir.AluOpType.max
PRELU = mybir.ActivationFunctionType.Prelu

_cache = {}


class _RR:
    """Round-robin dispatcher over eviction-capable engines."""

    def __init__(self, nc):
        self.nc = nc
        self.ev = 0          # evict cycle over [ACT, DVE, Pool]
        self.tr = 0          # tree/reduce cycle over [DVE, Pool]

    def evict(self, out, in_):
        """PSUM -> SBUF copy (+cast): GPSIMD cannot touch PSUM, so
        alternate the two PSUM-capable engines DVE / ACT."""
        k = self.ev % 2
        self.ev += 1
        if k == 0:
            self.nc.vector.tensor_copy(out, in_)
        else:
            self.nc.scalar.copy(out, in_)

    def lrelu(self, out, in_):
        """PSUM -> SBUF leaky-relu(0.2) eviction on ACT."""
        self.nc.scalar.activation(out, in_, PRELU, alpha=0.2)

    def chunk_sums(self, fea, dstv, tmpp):
        """8-lp block sums of fea [81, 16, 128] -> dstv [81, 2, 2, 64];
        binary add tree on Pool (SBUF-only engine)."""
        eng = self.nc.gpsimd
        t1 = tmpp.tile([P2, 8, 128], BF16, tag="t1", name="t1")
        t2 = tmpp.tile([P2, 4, 128], BF16, tag="t2", name="t2")
        f2 = fea[0:P2, :, :].rearrange("p (a two) b -> p a two b", two=2)
        eng.tensor_add(t1[0:P2, :, :], f2[:, :, 0, :], f2[:, :, 1, :])
        t1s = t1[0:P2, :, :].rearrange("p (a two) b -> p a two b", two=2)
        eng.tensor_add(t2[0:P2, :, :], t1s[:, :, 0, :], t1s[:, :, 1, :])
        t2s = t2[0:P2, :, :].rearrange(
            "p (g two) (s c) -> p g two s c", two=2, s=2)
        eng.tensor_add(dstv, t2s[:, :, 0, :, :], t2s[:, :, 1, :, :])


def _build_nc():
    nc = bacc.Bacc(None, target_bir_lowering=False, debug=False)
    yp_d = nc.declare_dram_parameter("yp", [NCHA, 128, NLPA, P2], BF16,
                                     isOutput=False)
    xp_d = nc.declare_dram_parameter("xp", [NCHB, 128, NLPB, P2], BF16,
                                     isOutput=False)
    wi_d = nc.declare_dram_parameter("wi", [128, 128], BF16, isOutput=False)
    wf_d = nc.declare_dram_parameter("wf", [128, 128], BF16, isOutput=False)
    wma_d = nc.declare_dram_parameter("wma", [82, P2], BF16, isOutput=False)
    wmb_d = nc.declare_dram_parameter("wmb", [82, P2], BF16, isOutput=False)
    bia1_d = nc.declare_dram_parameter("bia1", [1, NLPA * 128], BF16,
                                       isOutput=False)
    bia2_d = nc.declare_dram_parameter("bia2", [1, NLPA * 128], BF16,
                                       isOutput=False)
    bib_d = nc.declare_dram_parameter("bib", [1, NLPB * 128], BF16,
                                      isOutput=False)
    # out[ring, cgrp, bin_elem, ci, (slot, lp)]
    out_d = nc.declare_dram_parameter("out", [NRING, 8, P2, 8, 2 * RB], BF16,
                                      isOutput=True)
    pp_dram = nc.dram_tensor("ppd", [P2, 11, C], BF16)
    gath_d = nc.dram_tensor("gath", [NCORES, P2, 11, C], BF16,
                            addr_space="Shared")

    with nc.allow_low_precision("bf16 compute pipeline"), TileContext(nc) as tc:
        rr = _RR(nc)
        with (
            tc.tile_pool(name="const", bufs=1) as constp,
            tc.tile_pool(name="stage", bufs=6) as stp,
            tc.tile_pool(name="psc", bufs=2, space="PSUM") as pscp,
            tc.tile_pool(name="psm", bufs=2, space="PSUM") as psmp,
        ):
            wi_sb = constp.tile([128, 128], BF16, tag="wi")
            wf_sb = constp.tile([128, 128], BF16, tag="wf")
            wma_sb = constp.tile([82, P2], BF16, tag="wma")
            wmb_sb = constp.tile([82, P2], BF16, tag="wmb")
            # pooled[kk, bin, c] (bin-major so readback DMAs are contiguous)
            pooled = constp.tile([P2, NBIN, C], BF16, tag="pooled")
            nc.sync.dma_start(out=wi_sb[:, :], in_=wi_d[:, :])
            nc.sync.dma_start(out=wf_sb[:, :], in_=wf_d[:, :])
            nc.sync.dma_start(out=wma_sb[:, :], in_=wma_d[:, :])
            nc.sync.dma_start(out=wmb_sb[:, :], in_=wmb_d[:, :])

            def conv_mlp(src_d, ch, nlp, ms, w_sb, wm_sb, dst, dst_lp0):
                """conv+MLP+lrelu for chunk ch (nlp pairs); writes
                [81, nlp, 128] bf16 into dst at pair offset dst_lp0."""
                st = stp.tile([128, nlp, P2], BF16, tag="st", name="st")
                nc.sync.dma_start(out=st[:, :, :], in_=src_d[ch, :, :, :])
                for g in range(nlp // 8):
                    ps = pscp.tile([P2, 1024], F32, tag="psc", name="psc")
                    for j in range(8):
                        nc.tensor.matmul(
                            ps[0:P2, 128 * j:128 * (j + 1)],
                            st[:, 8 * g + j, :], w_sb[:, :],
                            start=True, stop=True,
                        )
                    d = ms[0:P2, 8 * g:8 * g + 8, :].rearrange("p a b -> p (a b)")
                    nc.vector.tensor_copy(d, ps[0:P2, :])
                flat = ms[:, 0:nlp, :].rearrange("p a b -> p (a b)")
                for q in range(nlp // 8):
                    mp = psmp.tile([P2, 1024], F32, tag="psm", name="psm")
                    nc.tensor.matmul(
                        mp[0:P2, 0:512], wm_sb[:, :],
                        flat[:, 1024 * q:1024 * q + 512],
                        start=True, stop=True,
                    )
                    nc.tensor.matmul(
                        mp[0:P2, 512:1024], wm_sb[:, :],
                        flat[:, 1024 * q + 512:1024 * (q + 1)],
                        start=True, stop=True,
                    )
                    dq = dst[0:P2, dst_lp0 + 8 * q:dst_lp0 + 8 * q + 8,
                             :].rearrange("p a b -> p (a b)")
                    rr.lrelu(dq, mp[0:P2, :])

            # ---------------- pass A: fea (y) + local pooling bins ----------
            with (
                tc.tile_pool(name="msap", bufs=1) as msap,
                tc.tile_pool(name="feap", bufs=2) as feap,
                tc.tile_pool(name="hsb", bufs=1) as hsbp,
            ):
                ms_a = [msap.tile([82, NLPA, 128], BF16, tag=f"msa{i}",
                                  name=f"msa{i}") for i in range(3)]
                for t in ms_a:
                    nc.sync.dma_start(
                        out=t[81:82, :, :].rearrange("p a b -> p (a b)"),
                        in_=bia1_d[:, :],
                    )
                # h_sb[kk, B, c]: B = 88*slot + h, staged-order 8-patch blocks
                h_sb = hsbp.tile([P2, 176, C], F32, tag="hsb")
                pp = hsbp.tile([P2, 11, C], BF16, tag="pp")
                for ch in range(NCHA):
                    if ch == 36:  # chunks 36-43 have slot1 pad columns
                        for t in ms_a:
                            nc.sync.dma_start(
                                out=t[81:82, :, :].rearrange("p a b -> p (a b)"),
                                in_=bia2_d[:, :],
                            )
                    fea = feap.tile([P2, NLPA, 128], BF16, tag="fea")
                    conv_mlp(yp_d, ch, NLPA, ms_a[ch % 3], wf_sb, wma_sb,
                             fea, 0)
                    # 8-lp block sums for both slots
                    dstv = h_sb[0:P2, :, :].rearrange(
                        "p (s B) c -> p B s c", s=2)[:, 2 * ch:2 * ch + 2, :, :]
                    rr.chunk_sums(fea, dstv, feap)
                # 11 bin sums: staged blocks [0:80)=bins0-4, [80:96)=bin5
                # (slot straddle), [96:176)=bins6-10 - same on every core.
                # DVE reduces the head while Pool trees the tail in parallel.
                nc.vector.tensor_reduce(
                    pp[0:P2, 0:5, :],
                    h_sb[0:P2, 0:80, :].rearrange("p (g r) c -> p g c r", r=16),
                    AX, ADD)
                tp1 = hsbp.tile([P2, 48, C], F32, tag="tp1")
                tp2 = hsbp.tile([P2, 24, C], F32, tag="tp2")
                tp3 = hsbp.tile([P2, 12, C], F32, tag="tp3")
                hv = h_sb[0:P2, 80:176, :].rearrange(
                    "p (a two) c -> p a two c", two=2)
                nc.gpsimd.tensor_add(tp1[0:P2, :, :], hv[:, :, 0, :],
                                     hv[:, :, 1, :])
                v1 = tp1[0:P2, :, :].rearrange("p (a two) c -> p a two c", two=2)
                nc.gpsimd.tensor_add(tp2[0:P2, :, :], v1[:, :, 0, :],
                                     v1[:, :, 1, :])
                v2 = tp2[0:P2, :, :].rearrange("p (a two) c -> p a two c", two=2)
                nc.gpsimd.tensor_add(tp3[0:P2, :, :], v2[:, :, 0, :],
                                     v2[:, :, 1, :])
                v3 = tp3[0:P2, :, :].rearrange("p (a two) c -> p a two c", two=2)
                nc.gpsimd.tensor_add(pp[0:P2, 5:11, :], v3[:, :, 0, :],
                                     v3[:, :, 1, :])
                nc.sync.dma_start(out=pp_dram[:, :, :], in_=pp[:, :, :])
                nc.gpsimd.collective_compute(
                    "AllGather",
                    mybir.AluOpType.bypass,
                    replica_groups=[list(range(NCORES))],
                    ins=[pp_dram[:, :, :]],
                    outs=[gath_d[:, :, :, :]],
                )

            # ---------------- pass B: img (x) + attention -------------------
            with (
                tc.tile_pool(name="msbp", bufs=1) as msbp,
                tc.tile_pool(name="imgp", bufs=2) as imgp,
                tc.tile_pool(name="attsb", bufs=2) as attp,
            ):
                ms_b = [msbp.tile([82, NLPB, 128], BF16, tag=f"msb{i}",
                                  name=f"msb{i}") for i in range(3)]
                for t in ms_b:
                    nc.sync.dma_start(
                        out=t[81:82, :, :].rearrange("p a b -> p (a b)"),
                        in_=bib_d[:, :],
                    )
                imgs = {}

                def ring_conv(r):
                    img = imgp.tile([P2, RB, 128], BF16, tag="img", name="img")
                    imgs[r] = img
                    for sub in range(RING):
                        ch = RING * r + sub
                        conv_mlp(xp_d, ch, NLPB, ms_b[ch % 3], wi_sb, wmb_sb,
                                 img, NLPB * sub)

                def ring_att(r):
                    img = imgs[r]
                    rhs_all = img[0:P2, :, :].rearrange(
                        "p l (s c) -> p c s l", s=2)
                    asb = None
                    for cp in range(C // 2):
                        ap = pscp.tile([P2, 1024], F32, tag="psc", name="psatt")
                        for i in range(2):
                            c = 2 * cp + i
                            nc.tensor.matmul(
                                ap[0:P2, 512 * i:512 * i + 2 * RB],
                                pooled[0:P2, :, c:c + 1], rhs_all[:, c, :, :],
                                start=True, stop=True,
                            )
                        if cp % 4 == 0:
                            asb = attp.tile([P2, 8, 2 * RB], BF16, tag="asb",
                                            name="asb")
                        src = ap[0:P2, :].rearrange(
                            "p (i x) -> p i x", i=2)[:, :, 0:2 * RB]
                        rr.evict(asb[0:P2, 2 * (cp % 4):2 * (cp % 4) + 2, :],
                                 src)
                        if cp % 4 == 3:
                            nc.sync.dma_start(
                                out=out_d[r, cp // 4, :, :, :],
                                in_=asb[0:P2, :, :],
                            )

                ring_conv(0)
                ring_conv(1)
                # pooled assembly: 8 fixed contiguous readback DMAs (core k
                # contributed global bins [10k, 10k+nb))
                for k in range(NCORES):
                    nb = 11 if k == NCORES - 1 else 10
                    nc.sync.dma_start(
                        out=pooled[0:P2, 10 * k:10 * k + nb, :],
                        in_=gath_d[k, :, 0:nb, :],
                    )
                ring_att(0)
                ring_conv(2)
                ring_att(1)
                ring_att(2)
    nc.compile()
    return nc


def _host_prep(x, y, w_img, b_img, w_fea, b_fea, w1, w2):
    f32 = np.float32
    bf16 = ml_dtypes.bfloat16
    weff = (w2.astype(np.float64) @ w1.astype(np.float64))  # (81, 81)
    wm = np.concatenate([weff.T, weff.sum(axis=1)[None, :]], axis=0).astype(f32)
    wma = (wm / 128.0).astype(bf16)   # fea pass: fold the 1/128 pool mean
    wmb = wm.astype(bf16)

    def pairw(w):
        blk = np.zeros((128, 128), dtype=f32)
        blk[0:64, 0:64] = w.T
        blk[64:128, 64:128] = w.T
        return blk.astype(bf16)

    wi = pairw(w_img.astype(f32))
    wf = pairw(w_fea.astype(f32))
    bia1 = np.tile(np.concatenate([b_fea, b_fea]).astype(f32), NLPA)[None, :]
    bia2_pad = np.tile(np.concatenate([b_fea, np.zeros_like(b_fea)])
                       .astype(f32), NLPA)[None, :]
    bib = np.tile(np.concatenate([b_img, b_img]).astype(f32), NLPB)[None, :]

    xf = np.asarray(x, dtype=f32).reshape(C, D, HW)
    yf = np.asarray(y, dtype=f32).reshape(C, D, HW)

    # global L-ordered patches [L, C, 81]
    def unf_global(t):
        return np.ascontiguousarray(
            t.reshape(C, NCORES, P, HW // P, P).transpose(1, 3, 0, 2, 4)
        ).reshape(L, C, P2)

    yg = unf_global(yf)
    # pass B: per-core D-row slab, slot-halved, chunk-major
    xg = unf_global(xf).reshape(NCORES, LLOC, C, P2)

    per_core = []
    for k in range(NCORES):
        # pass A staging: bin-aligned span + zero pads
        start = NREAL * k
        n_real = 2 * LPA if k == NCORES - 1 else NREAL
        arr = np.zeros((2 * LPA, C, P2), dtype=f32)
        arr[0:n_real] = yg[start:start + n_real]
        yp = (arr.reshape(2, LPA, C, P2).transpose(0, 2, 1, 3)
                 .reshape(128, NCHA, NLPA, P2).transpose(1, 0, 2, 3))
        # pass B staging: slot-halves, chunk-major
        xp = (xg[k].reshape(2, LPB, C, P2).transpose(0, 2, 1, 3)
                   .reshape(128, NCHB, NLPB, P2).transpose(1, 0, 2, 3))
        per_core.append({
            "yp": np.ascontiguousarray(yp).astype(bf16),
            "xp": np.ascontiguousarray(xp).astype(bf16),
            "wi": wi, "wf": wf, "wma": wma, "wmb": wmb,
            "bia1": bia1.astype(bf16),
            "bia2": (bia1 if k == NCORES - 1 else bia2_pad).astype(bf16),
            "bib": bib.astype(bf16),
        })
    return per_core


def kernel(x, y, w_img, b_img, w_fea, b_fea, w1, w2):
    if "nc" not in _cache:
        _cache["nc"] = _build_nc()
    nc = _cache["nc"]
    in_maps = _host_prep(x, y, w_img, b_img, w_fea, b_fea, w1, w2)
    trace = bool(os.environ.get("KERNEL_TRACE"))
    res = run_bass_kernel_spmd(nc, in_maps, list(range(NCORES)), trace=trace)
    _cache["last_result"] = res
    out = np.empty((1, C, D, H, W), dtype=np.float32)
    ov = out.reshape(C, D, HW)
    for k in range(NCORES):
        r = res.results[k]["out"].astype(np.float32)  # [3, 8, 81, 8, 432]
        r = r.reshape(NRING, 8, P2, 8, 2, RB)
        # att[m, c, l]; l = slot*648 + ring*216 + lp
        att = r.transpose(2, 1, 3, 4, 0, 5).reshape(P2, C, LLOC)
        blk = (att.transpose(1, 2, 0).reshape(C, LLOC, P, P)
                  .transpose(0, 2, 1, 3).reshape(C, P, HW))
        ov[:, P * k:P * (k + 1), :] = blk
    return out


# revision 34
# speedup vs baseline: 1.5387x; 1.0994x over previous
"""Trainium2 Bass kernel for nn_Cross_attention_3 (sparse_attention).

Two passes over 8 cores (uniform SPMD program):

Pass A (fea, y): each core owns a bin-ALIGNED span of 1408 global patches
(core k<7: patches [1280k, 1280k+1280) + 128 zero pads; core 7: the last
1408).  conv+MLP+lrelu, then 8-patch block sums (halves-paired add tree,
contiguous reads) and two grouped bin trees give this core's 11 local
128-patch pooling bins.  Zero pads contribute exactly 0 (zero data +
zero staged conv-bias columns, and lrelu(0)=0).  The 1/128 pool mean is
folded into the fea MLP weight (lrelu is positively homogeneous).  A
114KB-per-core bf16 AllGather exchanges the bins; 8 fixed readback DMAs
assemble the global pooled [81, 81bins, 64c] - identical constants on
every core.

Pass B (img, x): original even D-row sharding (1296 patches/core, no
pads), conv+MLP+lrelu into 3 rings of 216 pairs, then per-channel
attention matmuls against pooled.  The collective flies under ring 0/1
conv.

Engine budget: PE does matmuls only; DVE evicts conv PSUM, ACT evicts
MLP PSUM (lrelu); att evicts alternate DVE/ACT; Pool (no PSUM access on
TRN2) does all SBUF-side summation trees and issues the att out-DMA
descriptors.  conv emission is software-pipelined one chunk ahead of
MLP so the PE never waits on an eviction.
"""

import os
import sys

import numpy as np

try:
    import ml_dtypes
except ImportError:
    ml_dtypes = None

try:
    import concourse.bacc as _  # noqa: F401
except ImportError:  # container default path
    sys.path.insert(0, "/opt/trn_rl_repo")

import concourse.bacc as bacc
import concourse.mybir as mybir
from concourse.bass_utils import run_bass_kernel_spmd
from concourse.tile import TileContext

P = 9
P2 = 81
C = 64
D = 72
H = W = 108
HW = H * W
NCORES = 8
L = (D // P) * (HW // P)   # 10368 global patches
NBIN = 81                  # global 128-patch pooling bins

# pass A (fea): bin-aligned spans, 1408 staged patches = 704 pairs
LPA = 704
NLPA = 32
NCHA = LPA // NLPA         # 22 chunks
NREAL = 1280               # real patches for cores 0..6 (core 7: 1408)
BSW = 576 // NLPA          # 18: first chunk with slot1 pad columns

# pass B (img): even sharding, 1296 patches = 648 pairs
LLOC = HW // P             # 1296 patches per core
LPB = LLOC // 2            # 648
NLPB = 24
NCHB = LPB // NLPB         # 27 chunks
RING = 9                   # chunks per att ring
NRING = NCHB // RING       # 3
RB = RING * NLPB           # 216 pairs per ring

F32 = mybir.dt.float32
BF16 = mybir.dt.bfloat16
AX = mybir.AxisListType.X
ADD = mybir.AluOpType.add
PRELU = mybir.ActivationFunctionType.Prelu

_cache = {}


def _build_nc():
    nc = bacc.Bacc(None, target_bir_lowering=False, debug=False)
    yp_d = nc.declare_dram_parameter("yp", [NCHA, 128, NLPA, P2], BF16,
                                     isOutput=False)
    xp_d = nc.declare_dram_parameter("xp", [NCHB, 128, NLPB, P2], BF16,
                                     isOutput=False)
    wi_d = nc.declare_dram_parameter("wi", [128, 128], BF16, isOutput=False)
    wf_d = nc.declare_dram_parameter("wf", [128, 128], BF16, isOutput=False)
    wma_d = nc.declare_dram_parameter("wma", [82, P2], BF16, isOutput=False)
    wmb_d = nc.declare_dram_parameter("wmb", [82, P2], BF16, isOutput=False)
    bia1_d = nc.declare_dram_parameter("bia1", [1, NLPA * 128], BF16,
                                       isOutput=False)
    bia2_d = nc.declare_dram_parameter("bia2", [1, NLPA * 128], BF16,
                                       isOutput=False)
    bib_d = nc.declare_dram_parameter("bib", [1, NLPB * 128], BF16,
                                      isOutput=False)
    # out[ring, cgrp, bin_elem, ci, (slot, lp)]
    out_d = nc.declare_dram_parameter("out", [NRING, 16, P2, 4, 2 * RB], BF16,
                                      isOutput=True)
    pp_dram = nc.dram_tensor("ppd", [P2, 11, C], BF16)
    gath_d = nc.dram_tensor("gath", [NCORES, P2, 11, C], BF16,
                            addr_space="Shared")

    with nc.allow_low_precision("bf16 compute pipeline"), TileContext(nc) as tc:
        with (
            tc.tile_pool(name="const", bufs=1) as constp,
            tc.tile_pool(name="stage", bufs=6) as stp,
            tc.tile_pool(name="psc", bufs=2, space="PSUM") as pscp,
            tc.tile_pool(name="psm", bufs=2, space="PSUM") as psmp,
        ):
            wi_sb = constp.tile([128, 128], BF16, tag="wi")
            wf_sb = constp.tile([128, 128], BF16, tag="wf")
            wma_sb = constp.tile([82, P2], BF16, tag="wma")
            wmb_sb = constp.tile([82, P2], BF16, tag="wmb")
            # pooled[kk, bin, c] (bin-major so readback DMAs are contiguous)
            pooled = constp.tile([P2, NBIN, C], BF16, tag="pooled")
            # c-major copy so att LDWEIGHTS reads are contiguous
            pooled2 = constp.tile([P2, C, NBIN], BF16, tag="pooled2")
            nc.sync.dma_start(out=wi_sb[:, :], in_=wi_d[:, :])
            nc.sync.dma_start(out=wf_sb[:, :], in_=wf_d[:, :])
            nc.sync.dma_start(out=wma_sb[:, :], in_=wma_d[:, :])
            nc.sync.dma_start(out=wmb_sb[:, :], in_=wmb_d[:, :])

            def stage_chunk(src_d, ch, nlp, tag, nsplit=2):
                st = stp.tile([128, nlp, P2], BF16, tag=tag, name="st",
                              bufs=5 if tag == "stb" else None)
                step = nlp // nsplit
                for i in range(nsplit):
                    nc.sync.dma_start(
                        out=st[:, step * i:step * (i + 1), :],
                        in_=src_d[ch, :, step * i:step * (i + 1), :],
                    )
                return st

            def conv_part(st, nlp, ms, w_sb, act_evicts=0):
                """conv of staged chunk into ms rows 0:81; the last
                act_evicts PSUM groups are evicted on ACT instead of DVE."""
                ngr = nlp // 8
                for g in range(ngr):
                    ps = pscp.tile([P2, 1024], F32, tag="psc", name="psc")
                    for j in range(8):
                        nc.tensor.matmul(
                            ps[0:P2, 128 * j:128 * (j + 1)],
                            st[:, 8 * g + j, :], w_sb[:, :],
                            start=True, stop=True,
                        )
                    d = ms[0:P2, 8 * g:8 * g + 8, :].rearrange("p a b -> p (a b)")
                    if g >= ngr - act_evicts:
                        nc.scalar.copy(d, ps[0:P2, :])
                    else:
                        nc.vector.tensor_copy(d, ps[0:P2, :])

            def mlp_part(nlp, ms, wm_sb, dst, dst_lp0):
                """MLP + lrelu eviction into dst at pair offset dst_lp0."""
                flat = ms[:, 0:nlp, :].rearrange("p a b -> p (a b)")
                for q in range(nlp // 8):
                    mp = psmp.tile([P2, 1024], F32, tag="psm", name="psm")
                    nc.tensor.matmul(
                        mp[0:P2, 0:512], wm_sb[:, :],
                        flat[:, 1024 * q:1024 * q + 512],
                        start=True, stop=True,
                    )
                    nc.tensor.matmul(
                        mp[0:P2, 512:1024], wm_sb[:, :],
                        flat[:, 1024 * q + 512:1024 * (q + 1)],
                        start=True, stop=True,
                    )
                    dq = dst[0:P2, dst_lp0 + 8 * q:dst_lp0 + 8 * q + 8,
                             :].rearrange("p a b -> p (a b)")
                    nc.scalar.activation(dq, mp[0:P2, :], PRELU, alpha=0.2)

            stb_pre = {}
            # ---------------- pass A: fea (y) + local pooling bins ----------
            with (
                tc.tile_pool(name="msap", bufs=1) as msap,
                tc.tile_pool(name="feap", bufs=3) as feap,
                tc.tile_pool(name="hsb", bufs=1) as hsbp,
            ):
                ms_a = [msap.tile([82, NLPA, 128], BF16, tag=f"msa{i}",
                                  name=f"msa{i}") for i in range(3)]
                for t in ms_a:
                    nc.sync.dma_start(
                        out=t[81:82, :, :].rearrange("p a b -> p (a b)"),
                        in_=bia1_d[:, :],
                    )
                # h_sb[kk, B, c]: B = 88*slot + h, staged-order 8-patch blocks
                h_sb = hsbp.tile([P2, 176, C], F32, tag="hsb")
                pp = hsbp.tile([P2, 11, C], BF16, tag="pp")

                def tree_chunk(fea, ch):
                    """8-lp block sums (halves-paired, contiguous reads):
                    fea [81, 32, 128] -> h_sb blocks 4ch..4ch+3.  Whole tree
                    on one engine (no cross-engine hops); every 3rd chunk on
                    DVE, the rest on Pool."""
                    eng = nc.vector if ch % 3 == 2 else nc.gpsimd
                    t1 = feap.tile([P2, 4, 4, 128], BF16, tag="t1", name="t1")
                    t2 = feap.tile([P2, 4, 2, 128], BF16, tag="t2", name="t2")
                    f = fea[0:P2, :, :].rearrange(
                        "p (h two j) b -> p h two j b", two=2, j=4)
                    eng.tensor_add(t1[0:P2, :, :, :], f[:, :, 0, :, :],
                                   f[:, :, 1, :, :])
                    v1 = t1[0:P2, :, :, :].rearrange(
                        "p h (two j) b -> p h two j b", two=2)
                    eng.tensor_add(t2[0:P2, :, :, :], v1[:, :, 0, :, :],
                                   v1[:, :, 1, :, :])
                    v2 = t2[0:P2, :, :, :].rearrange(
                        "p h (two j) b -> p h two j b", two=2)
                    dstv = h_sb[0:P2, :, :].rearrange(
                        "p (s B) c -> p B s c", s=2)[:, 4 * ch:4 * ch + 4, :, :]
                    eng.tensor_add(dstv, v2[:, :, 0, 0, :].rearrange(
                        "p h (s c) -> p h s c", s=2),
                        v2[:, :, 1, 0, :].rearrange("p h (s c) -> p h s c", s=2))

                def bin_tree(eng, b0, nb, blk0, tag, dst):
                    """sum groups of 16 staged blocks [blk0, blk0+16*nb) into
                    dst bins (halves-paired, contiguous reads)."""
                    u1 = hsbp.tile([P2, nb, 8, C], BF16, tag=f"{tag}1")
                    u2 = hsbp.tile([P2, nb, 4, C], BF16, tag=f"{tag}2")
                    u3 = hsbp.tile([P2, nb, 2, C], BF16, tag=f"{tag}3")
                    hv = h_sb[0:P2, blk0:blk0 + 16 * nb, :].rearrange(
                        "p (g two j) c -> p g two j c", two=2, j=8)
                    eng.tensor_add(u1[0:P2, :, :, :], hv[:, :, 0, :, :],
                                   hv[:, :, 1, :, :])
                    w1 = u1[0:P2, :, :, :].rearrange(
                        "p g (two j) c -> p g two j c", two=2)
                    eng.tensor_add(u2[0:P2, :, :, :], w1[:, :, 0, :, :],
                                   w1[:, :, 1, :, :])
                    w2 = u2[0:P2, :, :, :].rearrange(
                        "p g (two j) c -> p g two j c", two=2)
                    eng.tensor_add(u3[0:P2, :, :, :], w2[:, :, 0, :, :],
                                   w2[:, :, 1, :, :])
                    w3 = u3[0:P2, :, :, :].rearrange(
                        "p g (two j) c -> p g two j c", two=2)
                    eng.tensor_add(dst, w3[:, :, 0, 0, :], w3[:, :, 1, 0, :])

                def a_conv(ch):
                    if BSW <= ch < BSW + 3:
                        # chunks >= BSW have slot1 pad columns; rewrite only
                        # this chunk's ms tile so earlier readers see bia1
                        t = ms_a[ch % 3]
                        nc.sync.dma_start(
                            out=t[81:82, :, :].rearrange("p a b -> p (a b)"),
                            in_=bia2_d[:, :],
                        )
                    st = stage_chunk(yp_d, ch, NLPA, "st",
                                     nsplit=8 if ch < 1 else 4 if ch < 3 else 2)
                    conv_part(st, NLPA, ms_a[ch % 3], wf_sb)

                a_conv(0)
                for ch in range(NCHA):
                    if ch + 1 < NCHA:
                        a_conv(ch + 1)
                    fea = feap.tile([P2, NLPA, 128], BF16, tag="fea")
                    mlp_part(NLPA, ms_a[ch % 3], wma_sb, fea, 0)
                    tree_chunk(fea, ch)
                    if ch == 19:
                        # staged blocks [0,80) complete -> bins 0-4 early
                        bin_tree(nc.vector, 0, 5, 0, "u", pp[0:P2, 0:5, :])
                        nc.sync.dma_start(out=pp_dram[:, 0:5, :],
                                          in_=pp[:, 0:5, :])
                        # pre-stage pass-B chunks so its conv starts
                        # the moment pass A drains
                        for bch in range(4):
                            stb_pre[bch] = stage_chunk(xp_d, bch, NLPB, "stb")
                # blocks [80,176) -> bins 5-10 (bin 5 straddles the slots)
                bin_tree(nc.gpsimd, 5, 6, 80, "v", pp[0:P2, 5:11, :])
                nc.sync.dma_start(out=pp_dram[:, 5:11, :], in_=pp[:, 5:11, :])
                nc.gpsimd.collective_compute(
                    "AllGather",
                    mybir.AluOpType.bypass,
                    replica_groups=[list(range(NCORES))],
                    ins=[pp_dram[:, :, :]],
                    outs=[gath_d[:, :, :, :]],
                )

            # ---------------- pass B: img (x) + attention -------------------
            with (
                tc.tile_pool(name="msbp", bufs=1) as msbp,
                tc.tile_pool(name="imgp", bufs=2) as imgp,
                tc.tile_pool(name="attsb", bufs=3) as attp,
            ):
                ms_b = [msbp.tile([82, NLPB, 128], BF16, tag=f"msb{i}",
                                  name=f"msb{i}") for i in range(3)]
                for t in ms_b:
                    nc.sync.dma_start(
                        out=t[81:82, :, :].rearrange("p a b -> p (a b)"),
                        in_=bib_d[:, :],
                    )
                imgs = {}

                def b_conv(ch):
                    st = stb_pre.pop(ch, None)
                    if st is None:
                        st = stage_chunk(xp_d, ch, NLPB, "stb")
                    conv_part(st, NLPB, ms_b[ch % 3], wi_sb)

                def ring_conv(r):
                    img = imgp.tile([P2, RB, 128], BF16, tag="img", name="img")
                    imgs[r] = img
                    ch0 = RING * r
                    if r == 0:
                        b_conv(ch0)
                    for sub in range(RING):
                        ch = ch0 + sub
                        if sub + 1 < RING:
                            b_conv(ch + 1)
                        elif r + 1 < NRING:
                            b_conv(ch + 1)  # first chunk of the next ring
                        mlp_part(NLPB, ms_b[ch % 3], wmb_sb, img, NLPB * sub)

                def ring_att(r):
                    img = imgs[r]
                    rhs_all = img[0:P2, :, :].rearrange(
                        "p l (s c) -> p c s l", s=2)
                    asb = None
                    for cp in range(C // 2):
                        # alternate pools: 4-deep att PSUM pipeline
                        pool = pscp if cp % 2 == 0 else psmp
                        ap = pool.tile([P2, 1024], F32,
                                       tag="psc" if cp % 2 == 0 else "psm",
                                       name="psatt")
                        for i in range(2):
                            c = 2 * cp + i
                            nc.tensor.matmul(
                                ap[0:P2, 512 * i:512 * i + 2 * RB],
                                pooled2[0:P2, c, :], rhs_all[:, c, :, :],
                                start=True, stop=True,
                            )
                        if cp % 2 == 0:
                            asb = attp.tile([P2, 4, 2 * RB], BF16, tag="asb",
                                            name="asb")
                        src = ap[0:P2, :].rearrange(
                            "p (i x) -> p i x", i=2)[:, :, 0:2 * RB]
                        dsta = asb[0:P2, 2 * (cp % 2):2 * (cp % 2) + 2, :]
                        if cp % 2 == 0:
                            nc.vector.tensor_copy(dsta, src)
                        else:
                            nc.scalar.copy(dsta, src)
                            # out-DMA descriptors ride the idle Pool queue
                            nc.gpsimd.dma_start(
                                out=out_d[r, cp // 2, :, 0:2, :],
                                in_=asb[0:P2, 0:2, :],
                            )
                            nc.gpsimd.dma_start(
                                out=out_d[r, cp // 2, :, 2:4, :],
                                in_=asb[0:P2, 2:4, :],
                            )

                ring_conv(0)
                ring_conv(1)
                # pooled assembly: 8 fixed contiguous readback DMAs (core k
                # contributed global bins [10k, 10k+nb))
                for k in range(NCORES):
                    nb = 11 if k == NCORES - 1 else 10
                    nc.sync.dma_start(
                        out=pooled[0:P2, 10 * k:10 * k + nb, :],
                        in_=gath_d[k, :, 0:nb, :],
                    )
                # 8-piece transpose so att channel c only waits on piece c//8
                for g in range(8):
                    nc.gpsimd.tensor_copy(
                        pooled2[0:P2, 8 * g:8 * g + 8, :],
                        pooled[0:P2, :, 8 * g:8 * g + 8].rearrange(
                            "p j c -> p c j"),
                    )
                ring_att(0)
                ring_conv(2)
                ring_att(1)
                ring_att(2)
    nc.compile()
    return nc


def _host_prep(x, y, w_img, b_img, w_fea, b_fea, w1, w2):
    f32 = np.float32
    bf16 = ml_dtypes.bfloat16
    weff = (w2.astype(np.float64) @ w1.astype(np.float64))  # (81, 81)
    wm = np.concatenate([weff.T, weff.sum(axis=1)[None, :]], axis=0).astype(f32)
    wma = (wm / 128.0).astype(bf16)   # fea pass: fold the 1/128 pool mean
    wmb = wm.astype(bf16)

    def pairw(w):
        blk = np.zeros((128, 128), dtype=f32)
        blk[0:64, 0:64] = w.T
        blk[64:128, 64:128] = w.T
        return blk.astype(bf16)

    wi = pairw(w_img.astype(f32))
    wf = pairw(w_fea.astype(f32))
    bia1 = np.tile(np.concatenate([b_fea, b_fea]).astype(f32), NLPA)[None, :]
    bia2_pad = np.tile(np.concatenate([b_fea, np.zeros_like(b_fea)])
                       .astype(f32), NLPA)[None, :]
    bib = np.tile(np.concatenate([b_img, b_img]).astype(f32), NLPB)[None, :]

    xf = np.asarray(x, dtype=f32).reshape(C, D, HW)
    yf = np.asarray(y, dtype=f32).reshape(C, D, HW)

    # global L-ordered patches [L, C, 81]
    def unf_global(t):
        return np.ascontiguousarray(
            t.reshape(C, NCORES, P, HW // P, P).transpose(1, 3, 0, 2, 4)
        ).reshape(L, C, P2)

    yg = unf_global(yf)
    xg = unf_global(xf).reshape(NCORES, LLOC, C, P2)

    per_core = []
    for k in range(NCORES):
        # pass A staging: bin-aligned span + zero pads
        start = NREAL * k
        n_real = 2 * LPA if k == NCORES - 1 else NREAL
        arr = np.zeros((2 * LPA, C, P2), dtype=f32)
        arr[0:n_real] = yg[start:start + n_real]
        yp = (arr.reshape(2, LPA, C, P2).transpose(0, 2, 1, 3)
                 .reshape(128, NCHA, NLPA, P2).transpose(1, 0, 2, 3))
        # pass B staging: slot-halves, chunk-major
        xp = (xg[k].reshape(2, LPB, C, P2).transpose(0, 2, 1, 3)
                   .reshape(128, NCHB, NLPB, P2).transpose(1, 0, 2, 3))
        per_core.append({
            "yp": np.ascontiguousarray(yp).astype(bf16),
            "xp": np.ascontiguousarray(xp).astype(bf16),
            "wi": wi, "wf": wf, "wma": wma, "wmb": wmb,
            "bia1": bia1.astype(bf16),
            "bia2": (bia1 if k == NCORES - 1 else bia2_pad).astype(bf16),
            "bib": bib.astype(bf16),
        })
    return per_core


def kernel(x, y, w_img, b_img, w_fea, b_fea, w1, w2):
    if "nc" not in _cache:
        _cache["nc"] = _build_nc()
    nc = _cache["nc"]
    in_maps = _host_prep(x, y, w_img, b_img, w_fea, b_fea, w1, w2)
    trace = bool(os.environ.get("KERNEL_TRACE"))
    res = run_bass_kernel_spmd(nc, in_maps, list(range(NCORES)), trace=trace)
    _cache["last_result"] = res
    out = np.empty((1, C, D, H, W), dtype=np.float32)
    ov = out.reshape(C, D, HW)
    for k in range(NCORES):
        r = res.results[k]["out"].astype(np.float32)  # [3, 16, 81, 4, 432]
        r = r.reshape(NRING, 16, P2, 4, 2, RB)
        # att[m, c, l]; l = slot*648 + ring*216 + lp
        att = r.transpose(2, 1, 3, 4, 0, 5).reshape(P2, C, LLOC)
        blk = (att.transpose(1, 2, 0).reshape(C, LLOC, P, P)
                  .transpose(0, 2, 1, 3).reshape(C, P, HW))
        ov[:, P * k:P * (k + 1), :] = blk
    return out
